# revision 8
# baseline (speedup 1.0000x reference)
"""Multi-head attention (B=2, S=2048, D=1024, H=16) on 8 Trainium2 NeuronCores.

Wall clock is dominated by the host<->device axon tunnel (~50 MB/s on
incompressible data, full-duplex capable, ~10ms per-buffer overhead), so the
design minimizes bytes on the wire and overlaps transfers:
  - fp16 wire format; all matmuls are fp16 x fp16 -> f32 psum (the PE array
    rounds f32 operands to ~tf32 anyway, so fp16 wire costs little accuracy;
    measured rel err ~8e-4 vs the f32 reference).
  - Weights/biases are uploaded once and kept device-resident; each call
    verifies the passed arrays still match the cached copy (identity check,
    then full memcmp) and re-uploads on any change.
  - Each kernel() call runs the 8-core program TWICE, once per batch; each
    core handles a 256-token query slice of that batch. The second batch's
    upload overlaps the first batch's execute/download (threaded fetch), so
    the duplex tunnel carries H2D and D2H concurrently.
  - Per run, each core uploads only its packed activation slice [3072,256]
    fp16 (1.5MB: q^T | k^T | v^T own-token columns) - zero duplication
    across cores. On device, an AllGather over all 8 cores reassembles the
    batch's full K/V token range; a second AllGather reassembles the full
    stacked weights [4096,1024] from the cached per-core 1/8 shards.
  - Donated output buffers are created ON DEVICE (jnp.zeros jit) one run
    ahead, so no output-sized zeros cross the tunnel.
  - The jax.jit(shard_map(bass_exec)) callable is built once and cached;
    run_bass_kernel_spmd would rebuild and re-trace it every call (~1s).

Device kernel (per core, per run):
  - Q projection for its own 256 tokens, K/V projections for the full 2048
    gathered tokens (all 16 heads), attention, output projection (+b_o),
    emitting out^T fp16 (0.5MB D2H per core per run).
  - Scores are computed transposed (S^T [k-tok, q-tok]) so softmax'd probs
    feed the PV matmul directly as the moving operand.
  - Softmax skips max-subtraction (scores/8 ~ N(0,1), exp can't overflow).
  - The per-head denominator l = sum_k exp(S) is produced by augmenting the
    PV stationary operand V with a ones-column (M=65): psum row 64 = l.
  - Normalization: linv = 1/l (DVE), broadcast across partitions with a
    K=1 ones-row matmul, then fused multiply during the PSUM->SBUF copy.
"""

import numpy as np
from concurrent.futures import ThreadPoolExecutor

D_MODEL = 1024
S = 2048
N_CORES = 8
SQ = S // N_CORES         # query tokens per core per run (256)
A = D_MODEL // 128        # 8 partition groups per 1024 features
N_HEADS = 16

R_Q = 0                   # packed activation tensor row offsets
R_KV = 1024
PK_ROWS = 3072            # qT 1024 + kT 1024 + vT 1024 (cols = own tokens)

C_BQ = 0                  # consts tensor row offsets ([8, 1024] fp16)
C_BK = 1
C_BO = 2
C_BV = 3
C_ONE = 4
CST_ROWS = 8

_STATE = None
_POOL = ThreadPoolExecutor(8)


def _build():
    from concourse import bacc
    import concourse.bass as bass
    import concourse.tile as tile
    from concourse import mybir

    F16 = mybir.dt.float16
    F32 = mybir.dt.float32
    EXP = mybir.ActivationFunctionType.Exp

    nc = bacc.Bacc("TRN2", target_bir_lowering=False, debug=False,
                   num_devices=N_CORES)

    pk = nc.dram_tensor("pk", [PK_ROWS, SQ], F16, kind="ExternalInput")
    wct = nc.dram_tensor("wct", [4 * D_MODEL // N_CORES, D_MODEL], F16,
                         kind="ExternalInput")
    cst = nc.dram_tensor("cst", [CST_ROWS, D_MODEL], F16, kind="ExternalInput")
    outT = nc.dram_tensor("outT", [D_MODEL, SQ], F16, kind="ExternalOutput")
    w_full = nc.dram_tensor("w_full", [4 * D_MODEL, D_MODEL], F16,
                            kind="Internal", addr_space="Shared")
    kvg = nc.dram_tensor("kvg", [N_CORES * 2 * D_MODEL, SQ], F16,
                         kind="Internal", addr_space="Shared")

    pk_ap = pk[:]
    cst_ap = cst[:]
    rg = [list(range(N_CORES))]

    with nc.allow_low_precision(reason="fp16 wire + matmul rounding intended"), \
            tile.TileContext(nc) as tc:
        with (
            tc.tile_pool(name="dram", bufs=1, space="DRAM") as dram,
            tc.tile_pool(name="wconst", bufs=1) as wconst,
            tc.tile_pool(name="big", bufs=1) as big,
            tc.tile_pool(name="xin", bufs=3) as xin_pool,
            tc.tile_pool(name="expp", bufs=4) as expp,
            tc.tile_pool(name="stage", bufs=3) as stage_pool,
            tc.tile_pool(name="bcp", bufs=2) as bcp,
            tc.tile_pool(name="small", bufs=4) as small,
            tc.tile_pool(name="psP", bufs=2, space="PSUM") as psP,
            tc.tile_pool(name="psS", bufs=2, space="PSUM") as psS,
            tc.tile_pool(name="psPo", bufs=2, space="PSUM") as psPo,
            tc.tile_pool(name="psB", bufs=1, space="PSUM") as psB,
        ):
            # ---- collectives: reassemble weights + this batch's kv ----
            kvb = dram.tile([2 * D_MODEL, SQ], F16)
            wb = dram.tile([4 * D_MODEL // N_CORES, D_MODEL], F16)
            nc.gpsimd.dma_start(wb[:], wct[:])
            nc.gpsimd.dma_start(
                kvb[:],
                bass.AP(tensor=pk_ap.tensor, offset=pk_ap.offset + R_KV * SQ,
                        ap=[[SQ, 2 * D_MODEL], [1, SQ]]))
            nc.gpsimd.collective_compute(
                "AllGather", mybir.AluOpType.bypass,
                replica_groups=rg, ins=[wb.opt()], outs=[w_full[:]],
            )
            nc.gpsimd.collective_compute(
                "AllGather", mybir.AluOpType.bypass,
                replica_groups=rg, ins=[kvb.opt()], outs=[kvg[:]],
            )

            # ---- constants / weights into SBUF ----
            # w_sb[p, w, a, f]: stacked weight w (q,k,v,o), in-dim = a*128+p.
            w_sb = wconst.tile([128, 4, A, D_MODEL], F16)
            nc.sync.dma_start(
                w_sb[:], w_full[:].rearrange("(w a p) f -> p w a f", w=4, a=A))
            bq_sb = wconst.tile([128, A], F32)
            bk_sb = wconst.tile([128, A], F32)
            bo_sb = wconst.tile([128, A], F32)
            bqkoh = wconst.tile([128, 3, A], F16)
            for i, row in enumerate((C_BQ, C_BK, C_BO)):
                nc.sync.dma_start(
                    bqkoh[:, i, :],
                    bass.AP(tensor=cst_ap.tensor,
                            offset=cst_ap.offset + row * D_MODEL,
                            ap=[[A, 128], [1, A]]))
            nc.vector.tensor_copy(bq_sb[:], bqkoh[:, 0, :])
            nc.vector.tensor_copy(bk_sb[:], bqkoh[:, 1, :])
            nc.vector.tensor_copy(bo_sb[:], bqkoh[:, 2, :])
            bv_bch = wconst.tile([128, N_HEADS, 64], F16)
            nc.gpsimd.dma_start(
                bv_bch[:],
                bass.AP(tensor=cst_ap.tensor,
                        offset=cst_ap.offset + C_BV * D_MODEL,
                        ap=[[0, 128], [64, N_HEADS], [1, 64]]))
            bv_bc = wconst.tile([128, N_HEADS, 64], F32)
            nc.vector.tensor_copy(bv_bc[:], bv_bch[:])
            ones_sb = wconst.tile([1, 64], F16)
            nc.sync.dma_start(
                ones_sb[:],
                bass.AP(tensor=cst_ap.tensor,
                        offset=cst_ap.offset + C_ONE * D_MODEL,
                        ap=[[0, 1], [1, 64]]))

            # ---- persistent activations ----
            QT_sb = big.tile([128, A, SQ], F16)        # Q^T[a*128+p, t_own]
            KT_sb = big.tile([128, A, S], F16)         # K^T[a*128+p, t]
            V_sb = big.tile([128, 16, N_HEADS, 65], F16)  # [t%128, t//128, h, c]
            OT_sb = big.tile([128, A, SQ], F16)        # attention out^T

            for tt in range(16):
                nc.gpsimd.dma_start(
                    V_sb[:, tt, :, 64:65],
                    bass.AP(tensor=cst_ap.tensor,
                            offset=cst_ap.offset + C_ONE * D_MODEL,
                            ap=[[0, 128], [0, N_HEADS], [1, 1]]))

            # ---- Q projection (own 256 tokens) ----
            xq_sb = xin_pool.tile([128, A, SQ], F16, tag="xin", name="xq")
            nc.sync.dma_start(
                xq_sb[:],
                bass.AP(tensor=pk_ap.tensor, offset=pk_ap.offset + R_Q * SQ,
                        ap=[[SQ, 128], [SQ * 128, A], [1, SQ]]))
            for ofb in range(A):
                pq = psP.tile([128, 512], F32, tag="pp", name=f"pq_{ofb}")
                for kt in range(A):
                    nc.tensor.matmul(
                        pq[:, 0:SQ], w_sb[:, 0, kt, ofb * 128:(ofb + 1) * 128],
                        xq_sb[:, kt, :],
                        start=(kt == 0), stop=(kt == A - 1),
                    )
                nc.vector.tensor_scalar_add(
                    QT_sb[:, ofb, :], pq[:, 0:SQ], bq_sb[:, ofb:ofb + 1])

            # ---- K and V projections (full 2048 tokens, gathered) ----
            kvg_ap = kvg[:].rearrange("(r k a p) t -> p r k a t",
                                      r=N_CORES, k=2, a=A)
            for r in range(N_CORES):
                kin = xin_pool.tile([128, A, SQ], F16, tag="xin", name=f"kin{r}")
                nc.sync.dma_start(kin[:], kvg_ap[:, r, 0, :, :])
                for ofb in range(A):
                    pk_ = psP.tile([128, 512], F32, tag="pp",
                                   name=f"pk_{r}_{ofb}")
                    for kt in range(A):
                        nc.tensor.matmul(
                            pk_[:, 0:SQ],
                            w_sb[:, 1, kt, ofb * 128:(ofb + 1) * 128],
                            kin[:, kt, :],
                            start=(kt == 0), stop=(kt == A - 1),
                        )
                    nc.vector.tensor_scalar_add(
                        KT_sb[:, ofb, r * SQ:(r + 1) * SQ], pk_[:, 0:SQ],
                        bk_sb[:, ofb:ofb + 1])
                vin = xin_pool.tile([128, A, SQ], F16, tag="xin", name=f"vin{r}")
                nc.sync.dma_start(vin[:], kvg_ap[:, r, 1, :, :])
                for tsub in range(SQ // 128):
                    tt = r * (SQ // 128) + tsub
                    for half in range(2):
                        pv = psP.tile([128, 512], F32, tag="pp",
                                      name=f"pv_{tt}_{half}")
                        for kt in range(A):
                            nc.tensor.matmul(
                                pv[:],
                                vin[:, kt, tsub * 128:(tsub + 1) * 128],
                                w_sb[:, 2, kt, half * 512:(half + 1) * 512],
                                start=(kt == 0), stop=(kt == A - 1),
                            )
                        nc.vector.tensor_add(
                            V_sb[:, tt, half * 8:(half + 1) * 8, 0:64],
                            pv[:].rearrange("p (h c) -> p h c", h=8),
                            bv_bc[:, half * 8:(half + 1) * 8, :])

            # ---- attention (16 heads x own 256 query tokens) ----
            for h in range(N_HEADS):
                p0 = (h % 2) * 64
                ofb = h // 2
                po = psPo.tile([128, SQ], F32, tag="po", name=f"po_{h}")
                for kt in range(16):
                    sc = psS.tile([128, SQ], F32, tag="sc", name=f"sc_{h}_{kt}")
                    nc.tensor.matmul(
                        sc[:],
                        KT_sb[p0:p0 + 64, ofb, kt * 128:(kt + 1) * 128],
                        QT_sb[p0:p0 + 64, ofb, :],
                        start=True, stop=True,
                        tile_position=(p0, 0),
                    )
                    ex = expp.tile([128, SQ], F16, tag="ex", name=f"ex_{h}_{kt}")
                    nc.scalar.activation(out=ex[:], in_=sc[:], func=EXP,
                                         scale=0.125)
                    nc.tensor.matmul(
                        po[0:65, :], V_sb[:, kt, h, :], ex[:],
                        start=(kt == 0), stop=(kt == 15),
                    )
                linv = small.tile([1, SQ], F16, tag="linv", name=f"linv_{h}")
                nc.vector.reciprocal(linv[:], po[64:65, :])
                bc_ps = psB.tile([64, SQ], F32, tag="bc", name=f"bc_{h}")
                nc.tensor.matmul(bc_ps[:], ones_sb[:], linv[:],
                                 start=True, stop=True)
                bc_sb = bcp.tile([64, SQ], F32, tag="bcs", name=f"bcs_{h}")
                nc.vector.tensor_copy(bc_sb[:], bc_ps[:])
                nc.vector.tensor_mul(
                    OT_sb[p0:p0 + 64, ofb, :], po[0:64, :], bc_sb[:])

            # ---- output projection (+b_o) ----
            for ofb in range(A):
                pg = psP.tile([128, 512], F32, tag="pp", name=f"pg_{ofb}")
                for ct in range(A):
                    nc.tensor.matmul(
                        pg[:, 0:SQ], w_sb[:, 3, ct, ofb * 128:(ofb + 1) * 128],
                        OT_sb[:, ct, :],
                        start=(ct == 0), stop=(ct == A - 1),
                    )
                st = stage_pool.tile([128, SQ], F16, tag="st", name=f"st_{ofb}")
                nc.vector.tensor_scalar_add(st[:], pg[:, 0:SQ],
                                            bo_sb[:, ofb:ofb + 1])
                nc.sync.dma_start(
                    outT[ofb * 128:(ofb + 1) * 128, :], st[:])

    nc.compile()
    return nc


class _State:
    pass


def _get_state():
    global _STATE
    if _STATE is None:
        import jax
        import jax.numpy as jnp
        from jax.sharding import Mesh, PartitionSpec, NamedSharding
        from jax.experimental.shard_map import shard_map
        from concourse import mybir
        from concourse.bass2jax import (
            _bass_exec_p, install_neuronx_cc_hook, partition_id_tensor)

        nc = _build()
        install_neuronx_cc_hook()
        partition_name = (nc.partition_id_tensor.name
                          if nc.partition_id_tensor else None)
        in_names, out_names, out_avals, zero_shapes = [], [], [], []
        for alloc in nc.m.functions[0].allocations:
            if not isinstance(alloc, mybir.MemoryLocationSet):
                continue
            name = alloc.memorylocations[0].name
            if alloc.kind == "ExternalInput":
                if name != partition_name:
                    in_names.append(name)
            elif alloc.kind == "ExternalOutput":
                shape = tuple(alloc.tensor_shape)
                dtype = mybir.dt.np(alloc.dtype)
                out_names.append(name)
                out_avals.append(jax.core.ShapedArray(shape, dtype))
                zero_shapes.append((shape, dtype))
        assert set(in_names) == {"pk", "wct", "cst"}, in_names
        assert out_names == ["outT"], out_names
        n_params = len(in_names)
        n_outs = len(out_avals)
        all_in_names = list(in_names) + list(out_names)
        if partition_name is not None:
            all_in_names.append(partition_name)

        def _body(*args):
            operands = list(args)
            if partition_name is not None:
                operands.append(partition_id_tensor())
            outs = _bass_exec_p.bind(
                *operands,
                out_avals=tuple(out_avals),
                in_names=tuple(all_in_names),
                out_names=tuple(out_names),
                lowering_input_output_aliases=(),
                sim_require_finite=True,
                sim_require_nnan=True,
                nc=nc,
            )
            return tuple(outs)

        devices = jax.devices()[:N_CORES]
        mesh = Mesh(np.asarray(devices), ("core",))
        st = _State()
        st.in_names = in_names
        st.sharding = NamedSharding(mesh, PartitionSpec("core"))
        st.sharded = jax.jit(
            shard_map(_body, mesh=mesh,
                      in_specs=(PartitionSpec("core"),) * (n_params + n_outs),
                      out_specs=(PartitionSpec("core"),) * n_outs,
                      check_rep=False),
            donate_argnums=tuple(range(n_params, n_params + n_outs)),
            keep_unused=True)
        zshapes = [(N_CORES * shp[0], *shp[1:]) for shp, dt in zero_shapes]
        zdtypes = [dt for shp, dt in zero_shapes]
        st.make_zeros = jax.jit(
            lambda: tuple(jnp.zeros(s, d) for s, d in zip(zshapes, zdtypes)),
            out_shardings=tuple(st.sharding for _ in zshapes))
        st.pending_zeros = []
        st.param_key = None
        st.w_dev = None
        st.cst_dev = None
        _STATE = st
    return _STATE


def _params_match(st, ws, bs):
    if st.param_key is None:
        return False
    kw, kb = st.param_key
    return (all(a is b or np.array_equal(a, b) for a, b in zip(kw, ws)) and
            all(a is b or np.array_equal(a, b) for a, b in zip(kb, bs)))


def _upload_params(st, ws, bs):
    import jax
    w_q, w_k, w_v, w_o = ws
    b_q, b_k, b_v, b_o = bs
    W16 = np.empty((4 * D_MODEL, D_MODEL), np.float16)
    W16[0:1024] = w_q
    W16[1024:2048] = w_k
    W16[2048:3072] = w_v
    W16[3072:4096] = w_o
    cst = np.zeros((CST_ROWS, D_MODEL), np.float16)
    cst[C_BQ] = np.ascontiguousarray(b_q.reshape(A, 128).T).reshape(-1)
    cst[C_BK] = np.ascontiguousarray(b_k.reshape(A, 128).T).reshape(-1)
    cst[C_BO] = np.ascontiguousarray(b_o.reshape(A, 128).T).reshape(-1)
    cst[C_BV] = b_v.astype(np.float16)
    cst[C_ONE, 0:64] = 1.0
    st.w_dev = jax.device_put(W16, st.sharding)
    st.cst_dev = jax.device_put(np.tile(cst, (N_CORES, 1)), st.sharding)
    st.param_key = (tuple(ws), tuple(bs))


def kernel(q, k, v, w_q, b_q, w_k, b_k, w_v, b_v, w_o, b_o):
    import jax

    st = _get_state()

    q, k, v = (np.asarray(x, np.float32) for x in (q, k, v))
    w_q, b_q, w_k, b_k, w_v, b_v, w_o, b_o = (
        np.asarray(x, np.float32)
        for x in (w_q, b_q, w_k, b_k, w_v, b_v, w_o, b_o)
    )
    ws, bs = (w_q, w_k, w_v, w_o), (b_q, b_k, b_v, b_o)
    if not _params_match(st, ws, bs):
        _upload_params(st, ws, bs)

    out = np.empty((2, S, D_MODEL), np.float32)
    pks = [np.empty((N_CORES, 3, D_MODEL, SQ), np.float16) for _ in range(2)]

    def pack(b, i, x):
        # pack x[b]^T own-token slices for all 8 cores in one strided pass
        np.copyto(pks[b][:, i],
                  x[b].reshape(N_CORES, SQ, D_MODEL).transpose(0, 2, 1),
                  casting="same_kind")

    while len(st.pending_zeros) < 2:
        st.pending_zeros.append(st.make_zeros())

    # All six tensors are packed by parallel workers, batch 0's jobs queued
    # first so its upload starts ASAP; batch 1 must be packed before batch
    # 0's upload finishes (~0.28s) or the tunnel goes idle.
    packs = [[_POOL.submit(pack, b, i, x) for i, x in enumerate((q, k, v))]
             for b in range(2)]
    fetches = []
    for b in range(2):
        for f in packs[b]:
            f.result()
        dev = jax.device_put(pks[b].reshape(N_CORES * PK_ROWS, SQ),
                             st.sharding)
        zeros = st.pending_zeros.pop(0)
        args = {"pk": dev, "wct": st.w_dev, "cst": st.cst_dev}
        outs = st.sharded(*[args[nm] for nm in st.in_names], *zeros)

        # fetch in a worker thread so this run's D2H overlaps the next H2D
        # (serializing the two fetches measured slightly worse)
        def fetch(o=outs, b=b):
            outT = np.asarray(o[0]).reshape(N_CORES, D_MODEL, SQ)
            np.copyto(out[b].reshape(N_CORES, SQ, D_MODEL),
                      outT.transpose(0, 2, 1), casting="same_kind")

        fetches.append(_POOL.submit(fetch))
    st.pending_zeros = [st.make_zeros(), st.make_zeros()]

    for f in fetches:
        f.result()
    return out
